# revision 4
# baseline (speedup 1.0000x reference)
"""Trainium2 Bass kernel for AIMQuantizerForVJEPA (residual VQ, 3 levels).

Math (forward pass):
  r0 = z @ W_in + b_in                      [BN, 256]
  per level l: score_l = r_l @ emb_l.T ; idx_l = argmax(score_l)
  (equivalent to argmin of squared distance since ||e_k|| ~= 1; the
   reference's ||e_k||^2 term varies only ~1e-7 so argmax(r.e) == argmin)
  z_q_l = emb_l[idx_l] ; r_{l+1} = r_l - z_q_l
  z_q_out = (z_q_0+z_q_1+z_q_2) @ W_out + b_out
  vq_loss = 0.25 * mean over levels of mean((z_q_l - r_l)^2)
          = 0.25/(3*BN*P) * (S1+S2+S3) where S_{l+1} = sum ||r_{l+1}||^2
  and the telescoping identity S_{l+1} = S_l - 2*sum_t max_t + sum_t ||e_sel||^2
  lets the host recover the loss from tiny device-side stashes.

Device-side trick: residuals for levels 1/2 are never materialized; the score
matmuls are corrected with host-precomputed Gram matrices
  score_1 = r0 @ emb1.T - G01[idx0]  with G01 = emb0 @ emb1.T
  score_2 = r0 @ emb2.T - G02[idx0] - G12[idx1]
where the row gathers are onehot matmuls (onehot from is_equal(score, max)).

Sharding: data-parallel over batch, 2 batches (4096 tokens) per core, 8 cores.
Everything on device is channel-on-partitions ([d|p, tokens]); the host
pre-transposes z and post-transposes z_q_out.
"""

import sys
import types

for _p in ("/opt/trn_rl_repo", "/opt/pypackages"):
    if _p not in sys.path:
        sys.path.insert(0, _p)

import numpy as np

import concourse.bass as bass
import concourse.mybir as mybir
import concourse.tile as tile
from concourse.bass import ts
from concourse.bass_utils import run_bass_kernel_spmd
from concourse.masks import make_identity

dt = mybir.dt
F32 = dt.float32
F32R = dt.float32r
AF = mybir.ActivationFunctionType
ALU = mybir.AluOpType
AX = mybir.AxisListType

B, N, D, P = 16, 2048, 1408, 256
KS = (64, 128, 256)
COMMIT = 0.25
NCORES = 8
TOK = B * N // NCORES        # 4096 tokens per core
NG = 8                       # groups per core
TG = TOK // NG               # 512 tokens per group
NT = TG // 128               # 4 tiles of 128 tokens per group
ND = D // 128                # 11 d-tiles
KOFF = (0, 64, 192)          # level col offsets in the 448-wide score panel
KTOT = sum(KS)               # 448

# dtype config: "f32" = exact fp32 (4 cyc/row), "f32r" = FP22 (1 cyc/row @N>=256)
CFG = {
    "proj": F32,     # z @ W_in         (feeds argmax -> exact)
    "score": F32,    # r0 @ embT        (feeds argmax -> exact)
    "gram": F32R,    # G corrections    (error ~4e-6 on scores, tolerable)
    "gather": F32R,  # onehot @ emb     (output-only)
    "outp": F32R,    # zq @ W_out       (output-only)
    "trans": F32,    # transpose out dtype must match in; keep f32
}


def _legalize_sync_waits(nc, limit=1):
    """This walrus build allows only one sync-wait per instruction; move
    excess waits onto dedicated NoOps just before their instruction."""
    import bass_rust

    n = 0
    for bb in nc.main_func.blocks:
        insts = list(bb.instructions)
        new = []
        changed = False
        for ins in insts:
            si = ins.sync_info
            waits = list(si.on_wait) if (si and si.on_wait) else []
            if len(waits) > limit:
                for w in waits[limit:]:
                    nop = bass_rust.InstNoOp(name=f"lgl-wait-{n}", ins=[], outs=[])
                    n += 1
                    nop.engine = ins.engine
                    nop.sync_info = bass_rust.SyncInfo(on_wait=[w], on_update=[])
                    new.append(nop)
                si.on_wait = waits[:limit]
                ins.sync_info = si
                changed = True
            new.append(ins)
        if changed:
            bb.instructions = new
    return n


def _build_nc():
    nc = bass.Bass()

    zt = nc.declare_dram_parameter("zt", [NG, D, TG], F32, isOutput=False)
    w_in = nc.declare_dram_parameter("w_in", [D, P], F32, isOutput=False)
    w_out = nc.declare_dram_parameter("w_out", [P, D], F32R, isOutput=False)
    embat = nc.declare_dram_parameter("embat", [P, KTOT], F32, isOutput=False)
    emb0 = nc.declare_dram_parameter("emb0", [KS[0], P], F32R, isOutput=False)
    emb1 = nc.declare_dram_parameter("emb1", [KS[1], P], F32R, isOutput=False)
    emb2 = nc.declare_dram_parameter("emb2", [KS[2], P], F32R, isOutput=False)
    ng01 = nc.declare_dram_parameter("ng01", [KS[0], KS[1]], F32R, isOutput=False)
    ng02 = nc.declare_dram_parameter("ng02", [KS[0], KS[2]], F32R, isOutput=False)
    ng12 = nc.declare_dram_parameter("ng12", [KS[1], KS[2]], F32R, isOutput=False)
    aux0 = nc.declare_dram_parameter("aux0", [KS[0], 2], F32R, isOutput=False)
    aux1 = nc.declare_dram_parameter("aux1", [KS[1], 2], F32R, isOutput=False)
    aux2 = nc.declare_dram_parameter("aux2", [KS[2], 2], F32R, isOutput=False)
    b_in = nc.declare_dram_parameter("b_in", [P, 1], F32, isOutput=False)
    b_out = nc.declare_dram_parameter("b_out", [D, 1], F32, isOutput=False)

    out_zq = nc.declare_dram_parameter("out_zq", [NG, D, TG], F32, isOutput=True)
    out_idx = nc.declare_dram_parameter("out_idx", [NG, 2, 3, TG], F32, isOutput=True)
    out_m = nc.declare_dram_parameter("out_m", [128, NG * NT * 3], F32, isOutput=True)
    out_ssq = nc.declare_dram_parameter("out_ssq", [128, NG * 2], F32, isOutput=True)

    with tile.TileContext(nc) as tc:
        import contextlib

        with contextlib.ExitStack() as ctx:
            singles = ctx.enter_context(tc.tile_pool(name="singles", bufs=1))
            ztp = ctx.enter_context(tc.tile_pool(name="ztp", bufs=2))
            grp = ctx.enter_context(tc.tile_pool(name="grp", bufs=2))
            ohp = ctx.enter_context(tc.tile_pool(name="ohp", bufs=2))
            outp = ctx.enter_context(tc.tile_pool(name="outp", bufs=2))
            psS = ctx.enter_context(tc.tile_pool(name="psS", bufs=2, space="PSUM"))
            psT = ctx.enter_context(tc.tile_pool(name="psT", bufs=1, space="PSUM"))
            psQ = ctx.enter_context(tc.tile_pool(name="psQ", bufs=1, space="PSUM"))
            psX = ctx.enter_context(tc.tile_pool(name="psX", bufs=1, space="PSUM"))
            psAD = ctx.enter_context(tc.tile_pool(name="psAD", bufs=2, space="PSUM"))

            # ---- constants ----
            ident = singles.tile([128, 128], F32)
            make_identity(nc, ident)
            w_in_sb = singles.tile([128, ND, P], F32)
            nc.sync.dma_start(out=w_in_sb, in_=w_in.rearrange("(c p) q -> p c q", p=128))
            w_out_sb = singles.tile([128, 2, D], F32R)
            nc.sync.dma_start(out=w_out_sb, in_=w_out.rearrange("(c p) q -> p c q", p=128))
            embat_sb = singles.tile([128, 2, KTOT], F32)
            nc.sync.dma_start(out=embat_sb, in_=embat.rearrange("(c p) q -> p c q", p=128))
            emb0_sb = singles.tile([KS[0], P], F32R)
            nc.sync.dma_start(out=emb0_sb, in_=emb0[:])
            emb1_sb = singles.tile([KS[1], P], F32R)
            nc.sync.dma_start(out=emb1_sb, in_=emb1[:])
            emb2_sb = singles.tile([128, 2, P], F32R)
            nc.sync.dma_start(out=emb2_sb, in_=emb2.rearrange("(c p) q -> p c q", p=128))
            ng01_sb = singles.tile([KS[0], KS[1]], F32R)
            nc.sync.dma_start(out=ng01_sb, in_=ng01[:])
            ng02_sb = singles.tile([KS[0], KS[2]], F32R)
            nc.sync.dma_start(out=ng02_sb, in_=ng02[:])
            ng12_sb = singles.tile([KS[1], KS[2]], F32R)
            nc.sync.dma_start(out=ng12_sb, in_=ng12[:])
            aux0_sb = singles.tile([KS[0], 2], F32R)
            nc.sync.dma_start(out=aux0_sb, in_=aux0[:])
            aux1_sb = singles.tile([KS[1], 2], F32R)
            nc.sync.dma_start(out=aux1_sb, in_=aux1[:])
            aux2_sb = singles.tile([128, 2, 2], F32R)
            nc.sync.dma_start(out=aux2_sb, in_=aux2.rearrange("(c p) q -> p c q", p=128))
            b_in_sb = singles.tile([128, 2], F32)
            nc.sync.dma_start(out=b_in_sb, in_=b_in.rearrange("(c p) q -> p (c q)", p=128))
            b_out_sb = singles.tile([128, ND], F32)
            nc.sync.dma_start(out=b_out_sb, in_=b_out.rearrange("(c p) q -> p (c q)", p=128))
            m_sb = singles.tile([128, NG * NT * 3], F32)
            ssq_sb = singles.tile([128, NG * 2], F32)
            dummy_sb = singles.tile([128, TG], F32)

            identr = ident

            for g in range(NG):
                zt_sb = ztp.tile([128, ND, TG], F32)
                nc.sync.dma_start(out=zt_sb, in_=zt[g].rearrange("(c p) t -> p c t", p=128))

                # ---- stage A: r0 = z @ W_in + b_in, channels on partitions ----
                r0_sb = grp.tile([128, 2, TG], F32)
                for pc in range(2):
                    ps = psAD.tile([128, TG], F32)
                    for di in range(ND):
                        nc.tensor.matmul(
                            ps,
                            lhsT=w_in_sb[:, di, ts(pc, 128)],
                            rhs=zt_sb[:, di, :],
                            start=(di == 0),
                            stop=(di == ND - 1),
                        )
                    nc.scalar.activation(
                        out=r0_sb[:, pc, :], in_=ps, func=AF.Identity,
                        bias=b_in_sb[:, pc : pc + 1],
                    )
                    # sum of squares of r0 (loss term S0)
                    nc.scalar.activation(
                        out=dummy_sb, in_=r0_sb[:, pc, :], func=AF.Square,
                        accum_out=ssq_sb[:, g * 2 + pc : g * 2 + pc + 1],
                    )

                # per-group transposed-onehot panels
                ohT0 = grp.tile([KS[0], TG], F32R)
                ohT1 = grp.tile([KS[1], TG], F32R)
                ohT2 = grp.tile([128, 2, TG], F32R)

                for i in range(NT):
                    tsl = ts(i, 128)
                    score = psS.tile([128, KTOT], F32)
                    for pc in range(2):
                        nc.tensor.matmul(
                            score,
                            lhsT=r0_sb[:, pc, tsl],
                            rhs=embat_sb[:, pc, :],
                            start=(pc == 0),
                            stop=False,
                        )
                    oh = ohp.tile([128, KTOT], F32)
                    trp = psT.tile([128, 4, 128], F32)

                    # level 0
                    mcol = g * NT * 3 + i * 3
                    nc.vector.tensor_reduce(
                        out=m_sb[:, mcol : mcol + 1], in_=score[:, 0 : KS[0]],
                        axis=AX.X, op=ALU.max,
                    )
                    nc.vector.tensor_scalar(
                        out=oh[:, 0 : KS[0]], in0=score[:, 0 : KS[0]],
                        scalar1=m_sb[:, mcol : mcol + 1], scalar2=None,
                        op0=ALU.is_equal,
                    )
                    nc.tensor.transpose(
                        trp[0 : KS[0], 0, :], oh[:, 0 : KS[0]],
                        identr,
                    )
                    nc.scalar.copy(out=ohT0[:, tsl], in_=trp[0 : KS[0], 0, :])
                    nc.tensor.matmul(
                        score[:, KOFF[1] : KOFF[1] + KS[1]],
                        lhsT=ohT0[:, tsl],
                        rhs=ng01_sb,
                        start=False, stop=False,
                    )
                    nc.tensor.matmul(
                        score[:, KOFF[2] :],
                        lhsT=ohT0[:, tsl],
                        rhs=ng02_sb,
                        start=False, stop=False,
                    )

                    # level 1
                    nc.vector.tensor_reduce(
                        out=m_sb[:, mcol + 1 : mcol + 2],
                        in_=score[:, KOFF[1] : KOFF[1] + KS[1]],
                        axis=AX.X, op=ALU.max,
                    )
                    nc.vector.tensor_scalar(
                        out=oh[:, KOFF[1] : KOFF[1] + KS[1]],
                        in0=score[:, KOFF[1] : KOFF[1] + KS[1]],
                        scalar1=m_sb[:, mcol + 1 : mcol + 2], scalar2=None,
                        op0=ALU.is_equal,
                    )
                    nc.tensor.transpose(
                        trp[:, 1, :],
                        oh[:, KOFF[1] : KOFF[1] + KS[1]],
                        identr,
                    )
                    nc.scalar.copy(out=ohT1[:, tsl], in_=trp[:, 1, :])
                    nc.tensor.matmul(
                        score[:, KOFF[2] :],
                        lhsT=ohT1[:, tsl],
                        rhs=ng12_sb,
                        start=False, stop=True,
                    )

                    # level 2
                    nc.vector.tensor_reduce(
                        out=m_sb[:, mcol + 2 : mcol + 3], in_=score[:, KOFF[2] :],
                        axis=AX.X, op=ALU.max,
                    )
                    nc.vector.tensor_scalar(
                        out=oh[:, KOFF[2] :], in0=score[:, KOFF[2] :],
                        scalar1=m_sb[:, mcol + 2 : mcol + 3], scalar2=None,
                        op0=ALU.is_equal,
                    )
                    for kc in range(2):
                        nc.tensor.transpose(
                            trp[:, 2 + kc, :],
                            oh[:, KOFF[2] + kc * 128 : KOFF[2] + (kc + 1) * 128],
                            identr,
                        )
                        nc.scalar.copy(out=ohT2[:, kc, tsl], in_=trp[:, 2 + kc, :])

                # ---- gather: zqcomb = sum_l onehot_l @ emb_l (f32r) ----
                zq_ps = psQ.tile([128, 2, TG], F32)
                gd = CFG["gather"]
                for pc in range(2):
                    nc.tensor.matmul(
                        zq_ps[:, pc, :], lhsT=emb0_sb[:, ts(pc, 128)],
                        rhs=ohT0, start=True, stop=False,
                    )
                    nc.tensor.matmul(
                        zq_ps[:, pc, :], lhsT=emb1_sb[:, ts(pc, 128)],
                        rhs=ohT1, start=False, stop=False,
                    )
                    for kc in range(2):
                        nc.tensor.matmul(
                            zq_ps[:, pc, :],
                            lhsT=emb2_sb[:, kc, ts(pc, 128)],
                            rhs=ohT2[:, kc, :],
                            start=False, stop=(kc == 1),
                        )
                zq_sb = grp.tile([128, 2, TG], F32R)
                for pc in range(2):
                    nc.scalar.copy(out=zq_sb[:, pc, :], in_=zq_ps[:, pc, :])

                # ---- aux: idx + ||e_sel||^2 rows via [iota, e2] columns ----
                aux_sb = grp.tile([2, 3, TG], F32)
                for lvl, (alhs, rhsT) in enumerate(
                    [(aux0_sb, [ohT0]), (aux1_sb, [ohT1]), (None, None)]
                ):
                    xps = psX.tile([2, TG], F32)
                    if lvl < 2:
                        nc.tensor.matmul(
                            xps, lhsT=alhs, rhs=rhsT[0],
                            start=True, stop=True,
                        )
                    else:
                        for kc in range(2):
                            nc.tensor.matmul(
                                xps, lhsT=aux2_sb[:, kc, :],
                                rhs=ohT2[:, kc, :],
                                start=(kc == 0), stop=(kc == 1),
                            )
                    nc.vector.tensor_copy(aux_sb[:, lvl, :], xps)
                nc.sync.dma_start(out=out_idx[g], in_=aux_sb)

                # ---- stage D: z_q_out^T = W_out^T @ zqcomb + b_out ----
                od = CFG["outp"]
                outT_sb = outp.tile([128, ND, TG], F32)
                for di in range(ND):
                    ps = psAD.tile([128, TG], F32)
                    for pc in range(2):
                        nc.tensor.matmul(
                            ps, lhsT=w_out_sb[:, pc, ts(di, 128)],
                            rhs=zq_sb[:, pc, :],
                            start=(pc == 0), stop=(pc == 1),
                        )
                    if di % 2 == 0:
                        nc.scalar.activation(
                            out=outT_sb[:, di, :], in_=ps, func=AF.Identity,
                            bias=b_out_sb[:, di : di + 1],
                        )
                    else:
                        nc.vector.tensor_scalar_add(
                            out=outT_sb[:, di, :], in0=ps,
                            scalar1=b_out_sb[:, di : di + 1],
                        )
                nc.sync.dma_start(
                    out=out_zq[g].rearrange("(c p) t -> p c t", p=128), in_=outT_sb
                )

            nc.sync.dma_start(out=out_m[:], in_=m_sb)
            nc.sync.dma_start(out=out_ssq[:], in_=ssq_sb)

    _legalize_sync_waits(nc)
    return nc


_NC_CACHE = None


def _get_nc():
    global _NC_CACHE
    if _NC_CACHE is None:
        _NC_CACHE = _build_nc()
    return _NC_CACHE


def _fp22(x):
    return (np.asarray(x, np.float32).view(np.int32) & np.int32(~0x3FF)).view(np.float32)


def kernel(z, W_in, b_in, W_out, b_out, emb0, emb1, emb2, _trace=False):
    z = np.asarray(z, np.float32)
    W_in = np.asarray(W_in, np.float32)
    b_in = np.asarray(b_in, np.float32)
    W_out = np.asarray(W_out, np.float32)
    b_out = np.asarray(b_out, np.float32)
    embs = [np.asarray(e, np.float32) for e in (emb0, emb1, emb2)]

    e64 = [e.astype(np.float64) for e in embs]
    embat = np.ascontiguousarray(
        np.concatenate([e.T for e in embs], axis=1)
    )  # [256, 448]
    ng01 = (-(e64[0] @ e64[1].T)).astype(np.float32)
    ng02 = (-(e64[0] @ e64[2].T)).astype(np.float32)
    ng12 = (-(e64[1] @ e64[2].T)).astype(np.float32)
    e2 = [np.sum(e * e, axis=1, dtype=np.float64) for e in e64]
    aux = [
        np.ascontiguousarray(
            np.stack([np.arange(k, dtype=np.float64), e2[l]], axis=1)
        ).astype(np.float32)
        for l, k in enumerate(KS)
    ]

    shared = {
        "w_in": W_in,
        "w_out": _fp22(W_out),
        "embat": embat,
        "emb0": _fp22(embs[0]),
        "emb1": _fp22(embs[1]),
        "emb2": _fp22(embs[2]),
        "ng01": _fp22(ng01),
        "ng02": _fp22(ng02),
        "ng12": _fp22(ng12),
        "aux0": _fp22(aux[0]),
        "aux1": _fp22(aux[1]),
        "aux2": _fp22(aux[2]),
        "b_in": np.ascontiguousarray(b_in.reshape(P, 1)),
        "b_out": np.ascontiguousarray(b_out.reshape(D, 1)),
    }
    bpc = B // NCORES
    in_maps = []
    for c in range(NCORES):
        zc = z[c * bpc : (c + 1) * bpc].reshape(TOK, D)
        ztb = np.ascontiguousarray(
            zc.T.reshape(D, NG, TG).transpose(1, 0, 2)
        )  # [NG, D, TG]
        in_maps.append({"zt": ztb, **shared})

    nc = _get_nc()
    res = run_bass_kernel_spmd(
        nc, in_maps, core_ids=list(range(NCORES)), trace=_trace
    )

    zq_full = np.empty((B, N, D), np.float32)
    idxs = [np.empty((B, N), np.int32) for _ in range(3)]
    S0 = 0.0
    M = np.zeros(3, np.float64)
    E = np.zeros(3, np.float64)
    for c in range(NCORES):
        r = res.results[c]
        zqb = r["out_zq"]  # [NG, D, TG]
        zq_full[c * bpc : (c + 1) * bpc] = (
            zqb.transpose(1, 0, 2).reshape(D, TOK).T.reshape(bpc, N, D)
        )
        ax = r["out_idx"]  # [NG, 2, 3, TG]
        for l in range(3):
            idxs[l][c * bpc : (c + 1) * bpc] = (
                np.rint(ax[:, 0, l, :]).astype(np.int32).reshape(bpc, N)
            )
            E[l] += ax[:, 1, l, :].astype(np.float64).sum()
        S0 += r["out_ssq"].astype(np.float64).sum()
        mst = r["out_m"].astype(np.float64)  # [128, NG*NT*3]
        mst = mst.reshape(128, NG * NT, 3)
        for l in range(3):
            M[l] += mst[:, :, l].sum()

    S1 = S0 - 2.0 * M[0] + E[0]
    S2 = S1 - 2.0 * M[1] + E[1]
    S3 = S2 - 2.0 * M[2] + E[2]
    vq_loss = np.float32(COMMIT * (S1 + S2 + S3) / (3.0 * B * N * P))

    out = (zq_full, idxs[0], idxs[1], idxs[2], vq_loss)
    if _trace:
        return out, res
    return out
